# revision 50
# baseline (speedup 1.0000x reference)
"""Trainium2 8-core tensor-parallel attention kernel (Bass/Tile), v3.

Sharding: heads tensor-parallel across 8 cores (2 heads/core) for
QKV + attention; output projection is column-sharded (each core owns
256 output channels) fed by per-chunk AllGathers of the pre-projection
attention outputs (2MB total exchanged vs 16.8MB for post-wo
ReduceScatter).

Single fused loop, interleaved to keep the PE p-state ramped:
  for sc in 0..7:                  # 512 flat seq rows; b = sc//4
    QKV q-chain for h=0
    normalize + AllGather of chunk sc-1   (softmax sums ready by now)
    rest of QKV (Q/K in [hd,seq] + RoPE, V in natural [seq,hd])
    attention q-group (scores/exp/AV; sums via in-place bf16 tree
    reduction on the contiguous probs buffer)
    O-projection of chunk sc-3            (AllGather long since done)

Self-contained: hardcodes B=2, S=2048, DIM=2048, NH=16, HD=128.
"""
import math

import numpy as np

B, S_FULL, DIM, NH = 2, 2048, 2048, 16
HD = 128
N_CORES = 8
HPC = NH // N_CORES          # heads per core (2)
OC = HPC * HD                # q/k/v channels per core (256)
OCD = DIM // N_CORES         # output channels per core (256)
DT = DIM // 128              # dim tiles (16)
SC_W = 512                   # schunk width (cols of flattened seq)

_CACHE = {}


def _build(S):
    """Build the 8-core SPMD Bass graph for sequence length S (B=2 fixed)."""
    import concourse.bass as bass
    import concourse.mybir as mybir
    import concourse.tile as tile
    from concourse import bacc

    fp32 = mybir.dt.float32
    bf16 = mybir.dt.bfloat16
    Exp = mybir.ActivationFunctionType.Exp
    Copy = mybir.ActivationFunctionType.Copy
    BYPASS = mybir.AluOpType.bypass

    FLAT = B * S                 # flattened rows (4096)
    NSC = FLAT // SC_W           # schunks / chunks (8)
    NQT = S // 128               # k-tiles per batch (16)
    SCALE = 1.0 / math.sqrt(HD)
    rg = [list(range(N_CORES))]

    nc = bacc.Bacc("TRN2", target_bir_lowering=False, debug=False,
                   num_devices=N_CORES)

    # ---- external parameters ----
    xt_d = nc.declare_dram_parameter("xt", [DIM, FLAT], bf16, isOutput=False)
    wqt_d = nc.declare_dram_parameter("wqt", [DIM, OC], bf16, isOutput=False)
    wkt_d = nc.declare_dram_parameter("wkt", [DIM, OC], bf16, isOutput=False)
    wvt_d = nc.declare_dram_parameter("wvt", [DIM, OC], bf16, isOutput=False)
    wot_d = nc.declare_dram_parameter("wotc", [DIM, OCD], bf16, isOutput=False)
    cos_d = nc.declare_dram_parameter("cos_t", [HD, S], bf16, isOutput=False)
    sin_d = nc.declare_dram_parameter("sin_t", [HD, S], bf16, isOutput=False)
    mdg_d = nc.declare_dram_parameter("mask_diag", [NQT, 128, 128], bf16, isOutput=False)
    rot_d = nc.declare_dram_parameter("rotp", [128, 128], bf16, isOutput=False)
    on2_d = nc.declare_dram_parameter("ones128", [128, 128], bf16, isOutput=False)
    out_d = nc.declare_dram_parameter("outT", [OCD, FLAT], bf16, isOutput=True)

    # ---- internal DRAM (collective staging) ----
    ag_in_d = [nc.dram_tensor(f"ag_in{p}", [OC, SC_W], bf16) for p in range(NSC)]
    ag_out_d = [nc.dram_tensor(f"ag_out{p}", [N_CORES * OC, SC_W], bf16,
                               addr_space="Shared") for p in range(NSC)]
    warm_in_d = nc.dram_tensor("warm_in", [8, 16], bf16)
    warm_out_d = nc.dram_tensor("warm_out", [64, 16], bf16, addr_space="Shared")
    # per-head AllGather buffers for the final chunk (1MB each, so the h0
    # exchange overlaps the h1 attention section)
    ag7h_d = [nc.dram_tensor(f"ag7h{h}", [N_CORES * 128, SC_W], bf16,
                             addr_space="Shared") for h in range(HPC)]

    from contextlib import ExitStack
    with tile.TileContext(nc) as tc:
        with ExitStack() as _stk:
            cpool = _stk.enter_context(tc.tile_pool(name="consts", bufs=1))
            wpool = _stk.enter_context(tc.tile_pool(name="wqkv", bufs=1))
            xpool = _stk.enter_context(tc.tile_pool(name="xT", bufs=34))
            qkpool = _stk.enter_context(tc.tile_pool(name="qk_sb", bufs=1))
            vpool = _stk.enter_context(tc.tile_pool(name="vbf", bufs=1))
            spool = _stk.enter_context(tc.tile_pool(name="p1tmp", bufs=3))
            tpool = _stk.enter_context(tc.tile_pool(name="t1tmp", bufs=2))
            ptpool = _stk.enter_context(tc.tile_pool(name="probsT", bufs=1))
            smpool = _stk.enter_context(tc.tile_pool(name="small", bufs=2))
            aglpool = _stk.enter_context(tc.tile_pool(name="agl", bufs=17))
            obpool = _stk.enter_context(tc.tile_pool(name="outsb", bufs=3))
            qkvps = _stk.enter_context(tc.tile_pool(name="qkvps", bufs=2, space="PSUM"))
            rotps = _stk.enter_context(tc.tile_pool(name="rotps", bufs=1, space="PSUM"))
            scps = _stk.enter_context(tc.tile_pool(name="scps", bufs=2, space="PSUM"))
            pops = _stk.enter_context(tc.tile_pool(name="pops", bufs=2, space="PSUM"))
            opps = _stk.enter_context(tc.tile_pool(name="opps", bufs=1, space="PSUM"))

            # warm-up collective: absorbs the cold-start mesh setup (~40-70us
            # on the first collective) behind the DMA head phase
            nc.gpsimd.collective_compute(
                "AllGather", BYPASS, replica_groups=rg,
                ins=[warm_in_d[:]], outs=[warm_out_d[:]])

            # ---- weights + first x tiles interleaved (critical path) ----
            # per-dt weight tiles so the first chains start as soon as their
            # first slabs land, not after the whole 1MB tensor
            w_sb = {nm: [] for nm in ("q", "k", "v")}
            for nm, d in (("q", wqt_d), ("k", wkt_d), ("v", wvt_d)):
                for dt in range(DT):
                    w = wpool.tile([128, OC], bf16, tag=f"w{nm}{dt}",
                                   name=f"w{nm}{dt}")
                    w_sb[nm].append(w)
            xts0 = []
            for dt in range(DT):
                nc.sync.dma_start(w_sb["q"][dt][:],
                                  wqt_d[dt * 128:(dt + 1) * 128, :])
                xt = xpool.tile([128, SC_W], bf16, tag="xt", name=f"xt{dt}")
                nc.sync.dma_start(xt[:], xt_d[dt * 128:(dt + 1) * 128, 0:SC_W])
                xts0.append(xt)
            for dt in range(DT):
                nc.sync.dma_start(w_sb["k"][dt][:],
                                  wkt_d[dt * 128:(dt + 1) * 128, :])
            for dt in range(DT):
                nc.sync.dma_start(w_sb["v"][dt][:],
                                  wvt_d[dt * 128:(dt + 1) * 128, :])

            # persistent SBUF tensors
            qTa, kTa, vbfa = {}, {}, {}
            for bb in range(B):
                for h in range(HPC):
                    qTa[(bb, h)] = qkpool.tile([128, S], bf16, tag=f"qT{bb}{h}",
                                               name=f"qT{bb}{h}")
                    kTa[(bb, h)] = qkpool.tile([128, S], bf16, tag=f"kT{bb}{h}",
                                               name=f"kT{bb}{h}")
                    vbfa[(bb, h)] = vpool.tile([128, NQT, HD], bf16,
                                               tag=f"v{bb}{h}", name=f"v{bb}{h}")

            cos_sb = cpool.tile([HD, S], bf16)
            sin_sb = cpool.tile([HD, S], bf16)
            mdg_sb = cpool.tile([128, NQT, 128], bf16)
            rot_sb = cpool.tile([128, 128], bf16)
            on2_sb = cpool.tile([128, 128], bf16)
            wot_sb = cpool.tile([128, DT, OCD], bf16)

            def load_consts():
                nc.sync.dma_start(cos_sb[:], cos_d[:])
                nc.sync.dma_start(sin_sb[:], sin_d[:])
                nc.sync.dma_start(mdg_sb[:], mdg_d[:].rearrange("t p k -> p t k"))
                nc.sync.dma_start(rot_sb[:], rot_d[:])
                nc.sync.dma_start(on2_sb[:], on2_d[:])
                for dt in range(DT):
                    nc.sync.dma_start(wot_sb[:, dt, :],
                                      wot_d[dt * 128:(dt + 1) * 128, :])

            # state carried between loop iterations for deferred normalize
            pending = {}   # chunk p -> (qg, b, {h: (pt_buf, po_ps)})
            # pending O-proj work quanta (generators), pumped between
            # attention kt-steps to fill the PE's exp-wait micro-gaps
            op_queue = []

            def pump(n):
                k = 0
                while op_queue and k < n:
                    try:
                        next(op_queue[0])
                        k += 1
                    except StopIteration:
                        op_queue.pop(0)

            def flush_ops():
                while op_queue:
                    try:
                        next(op_queue[0])
                    except StopIteration:
                        op_queue.pop(0)

            # ============ per-chunk attention (h sections only) ============
            def attn_chunk(qg, b, norm_inline=False):
                kmax = qg * 4 + 3
                K = kmax + 1
                p = b * 4 + qg
                hstate = {}
                for h in range(HPC):
                    po_ps = pops.tile([128, SC_W], fp32, tag="po", name=f"po{h}")
                    ptb = ptpool.tile([128, NQT, SC_W], bf16, tag=f"pTb{h}",
                                      name=f"pTb{h}")
                    # zero the above-diagonal cols of the partial tiles so the
                    # tree reduction sees exact zeros there
                    for kt in range(qg * 4 + 1, kmax + 1):
                        qlo = (kt - qg * 4) * 128
                        nc.gpsimd.memset(ptb[:, kt, 0:qlo], 0)
                    for kt in range(K):
                        qlo = max(0, kt - qg * 4) * 128
                        n = SC_W - qlo
                        sp = scps.tile([128, SC_W], fp32, tag="sc", name="sp")
                        nc.tensor.matmul(
                            sp[:, :n],
                            kTa[(b, h)][:, kt * 128:(kt + 1) * 128],
                            qTa[(b, h)][:, qg * SC_W + qlo:(qg + 1) * SC_W],
                            start=True, stop=True)
                        if kt >= qg * 4:
                            nc.vector.tensor_add(
                                sp[:, 0:128], sp[:, 0:128], mdg_sb[:, kt, :])
                        nc.scalar.activation(ptb[:, kt, qlo:SC_W], sp[:, :n], Exp)
                        if kt >= 1:
                            kl = kt - 1
                            ql2 = max(0, kl - qg * 4) * 128
                            nc.tensor.matmul(
                                po_ps[:, ql2:SC_W], vbfa[(b, h)][:, kl, :],
                                ptb[:, kl, ql2:SC_W],
                                start=(kl == 0), stop=False)
                        pump(2)
                    ql2 = max(0, kmax - qg * 4) * 128
                    nc.tensor.matmul(
                        po_ps[:, ql2:SC_W], vbfa[(b, h)][:, kmax, :],
                        ptb[:, kmax, ql2:SC_W], start=(kmax == 0), stop=True)
                    # in-place bf16 tree reduction over the kt axis -> ptb[:,0,:]
                    kk = K
                    while kk > 1:
                        m = kk // 2
                        nc.vector.tensor_add(ptb[:, 0:m, :], ptb[:, 0:m, :],
                                             ptb[:, kk - m:kk, :])
                        kk -= m
                    if norm_inline:
                        # final chunk: normalize + 1MB per-head AllGather right
                        # after this head's section, so the h0 exchange runs
                        # under the h1 attention compute
                        pump(8)
                        normalize_h(p, h, ptb, po_ps)
                        nc.gpsimd.collective_compute(
                            "AllGather", BYPASS, replica_groups=rg,
                            ins=[ag_in_d[p][h * 128:(h + 1) * 128, :]],
                            outs=[ag7h_d[h][:]])
                    hstate[h] = (ptb, po_ps)
                if not norm_inline:
                    pending[p] = (qg, b, hstate)

            # ============ deferred normalize + AllGather ============
            def normalize_h(p, h, ptb, po_ps):
                sb_ps = rotps.tile([128, SC_W], fp32, tag="rot", name="sb_ps")
                nc.tensor.matmul(sb_ps[:], on2_sb[:], ptb[:, 0, :],
                                 start=True, stop=True)
                rbc = smpool.tile([128, SC_W], fp32, tag="rbc", name="rbc")
                nc.vector.reciprocal_approx_fast(rbc[:], sb_ps[:])
                ob = smpool.tile([128, SC_W], bf16, tag="obuf", name="ob")
                nc.vector.tensor_mul(ob[:], po_ps[:], rbc[:])
                nc.sync.dma_start(ag_in_d[p][h * 128:(h + 1) * 128, :], ob[:])

            def normalize_chunk(p):
                qg, b, hstate = pending.pop(p)
                for h in range(HPC):
                    ptb, po_ps = hstate[h]
                    normalize_h(p, h, ptb, po_ps)
                nc.gpsimd.collective_compute(
                    "AllGather", BYPASS, replica_groups=rg,
                    ins=[ag_in_d[p][:]], outs=[ag_out_d[p][:]])

            # ============ per-chunk O-projection (column-sharded) ============
            def oproj_gen(p):
                """Generator: one yield per matmul so the caller can weave
                the chain between attention steps."""
                slabs = []
                for e in range(DT):
                    agl = aglpool.tile([128, SC_W], bf16, tag="agl",
                                       name=f"agl{e}")
                    nc.sync.dma_start(agl[:],
                                      ag_out_d[p][e * 128:(e + 1) * 128, :])
                    slabs.append(agl)
                for dh in range(2):
                    # dh=1 borrows the rot pool's bank so the two half-chains
                    # don't serialize on a single PSUM drain
                    pool = opps if dh == 0 else rotps
                    op_ps = pool.tile([128, SC_W], fp32,
                                      tag="op" if dh == 0 else "rot",
                                      name="op_ps")
                    for e in range(DT):
                        nc.tensor.matmul(
                            op_ps[:],
                            wot_sb[:, e, dh * 128:(dh + 1) * 128],
                            slabs[e][:],
                            start=(e == 0), stop=(e == DT - 1))
                        yield
                    obt = obpool.tile([128, SC_W], bf16, tag="ob", name="obt")
                    if dh == 0:
                        nc.scalar.copy(obt[:], op_ps[:])
                    else:
                        nc.vector.tensor_copy(obt[:], op_ps[:])
                    nc.sync.dma_start(
                        out_d[dh * 128:(dh + 1) * 128,
                              p * SC_W:(p + 1) * SC_W], obt[:])

            def oproj_chunk(p):
                for _ in oproj_gen(p):
                    pass

            def oproj7_split():
                """Final chunk O-proj from the two per-head AllGathers; even
                (h0) head slabs are consumed first so the PE runs while the
                h1 AllGather is still in flight."""
                p = NSC - 1
                order = list(range(0, NH // N_CORES * 8, 2)) + \
                    list(range(1, NH // N_CORES * 8, 2))
                slabs = {}
                for g in order:
                    agl = aglpool.tile([128, SC_W], bf16, tag="agl",
                                       name=f"agl{g}")
                    nc.sync.dma_start(
                        agl[:], ag7h_d[g % 2][(g // 2) * 128:
                                              (g // 2 + 1) * 128, :])
                    slabs[g] = agl
                for dh in range(2):
                    pool = opps if dh == 0 else rotps
                    op_ps = pool.tile([128, SC_W], fp32,
                                      tag="op" if dh == 0 else "rot",
                                      name="op_ps")
                    for i, g in enumerate(order):
                        nc.tensor.matmul(
                            op_ps[:],
                            wot_sb[:, g, dh * 128:(dh + 1) * 128],
                            slabs[g][:],
                            start=(i == 0), stop=(i == DT - 1))
                    obt = obpool.tile([128, SC_W], bf16, tag="ob", name="obt")
                    if dh == 0:
                        nc.scalar.copy(obt[:], op_ps[:])
                    else:
                        nc.vector.tensor_copy(obt[:], op_ps[:])
                    nc.sync.dma_start(
                        out_d[dh * 128:(dh + 1) * 128,
                              p * SC_W:(p + 1) * SC_W], obt[:])

            # ============ main fused loop ============
            carried_xts = None
            for sc in range(NSC):
                b, qg = divmod(sc, 4)
                c0 = qg * SC_W               # column offset within batch
                xts = xts0 if sc == 0 else carried_xts

                def emit_rope_mm(t, h, til):
                    rp = rotps.tile([128, SC_W], fp32, tag="rot", name="rp")
                    nc.tensor.matmul(rp[:], rot_sb[:], til[:],
                                     start=True, stop=True)
                    dst = (qTa if t == "q" else kTa)[(b, h)]
                    t1 = tpool.tile([128, SC_W], bf16, tag="t1", name="t1")
                    nc.gpsimd.tensor_mul(t1[:], til[:], cos_sb[:, c0:c0 + SC_W])
                    hat = spool.tile([128, SC_W], bf16, tag="hat", name="hat")
                    nc.vector.tensor_mul(hat[:], rp[:], sin_sb[:, c0:c0 + SC_W])
                    nc.vector.tensor_add(dst[:, c0:c0 + SC_W], hat[:], t1[:])

                def v_gen(vsc, vxts):
                    """Natural-layout V chains for schunk vsc (stationary =
                    x tile, both heads at once). No scalar inputs, so these
                    weave safely into exp-bound attention sections."""
                    vb, vqg = divmod(vsc, 4)
                    for vt in range(4):
                        psv = qkvps.tile([128, OC], fp32, tag="qkv",
                                         name="ps_v")
                        for dt in range(DT):
                            nc.tensor.matmul(
                                psv[:],
                                vxts[dt][:, vt * 128:(vt + 1) * 128],
                                w_sb["v"][dt][:],
                                start=(dt == 0), stop=(dt == DT - 1))
                            yield
                        for hh in range(HPC):
                            if hh == 0:
                                nc.scalar.copy(
                                    vbfa[(vb, hh)][:, vqg * 4 + vt, :],
                                    psv[:, hh * HD:(hh + 1) * HD])
                            else:
                                nc.vector.tensor_copy(
                                    vbfa[(vb, hh)][:, vqg * 4 + vt, :],
                                    psv[:, hh * HD:(hh + 1) * HD])

                first_chain_done = False
                for h in range(HPC):
                    tils = {}
                    for t in ("q", "k"):
                        ps = qkvps.tile([128, SC_W], fp32, tag="qkv",
                                        name=f"ps_{t}")
                        for dt in range(DT):
                            nc.tensor.matmul(
                                ps[:],
                                w_sb[t][dt][:, h * HD:(h + 1) * HD],
                                xts[dt][:],
                                start=(dt == 0), stop=(dt == DT - 1))
                        if not first_chain_done:
                            first_chain_done = True
                            if sc == 0:
                                load_consts()
                            if sc >= 1:
                                # normalize previous chunk while this chunk's
                                # q-chain covers the PE
                                normalize_chunk(sc - 1)
                        til = spool.tile([128, SC_W], bf16, tag="til",
                                         name=f"til_{t}")
                        if t == "q":
                            nc.scalar.activation(til[:], ps[:], Copy,
                                                 scale=SCALE)
                        else:
                            nc.scalar.copy(til[:], ps[:])
                        tils[t] = til
                        if t == "k":
                            emit_rope_mm("q", h, tils["q"])
                    if h == 0 and sc == 0:
                        # sc 0's V runs inline; later scs' V is woven into
                        # the previous chunk's attention
                        for _ in v_gen(0, xts):
                            pass
                    emit_rope_mm("k", h, tils["k"])

                # prefetch next schunk's x tiles for the woven V chains
                if sc + 1 < NSC:
                    nxts = []
                    for dt in range(DT):
                        xt = xpool.tile([128, SC_W], bf16, tag="xt",
                                        name=f"xt{dt}")
                        nc.sync.dma_start(
                            xt[:], xt_d[dt * 128:(dt + 1) * 128,
                                        (sc + 1) * SC_W:(sc + 2) * SC_W])
                        nxts.append(xt)
                    next_xts = nxts
                else:
                    next_xts = None
                carried_xts = next_xts

                # weave next schunk's V and earlier chunks' O-proj through
                # this chunk's attention (exp-bound) section
                if next_xts is not None:
                    op_queue.append(v_gen(sc + 1, next_xts))
                if sc >= 4:
                    op_queue.append(oproj_gen(sc - 4))
                if sc == NSC - 1:
                    # extra weave fodder: chunk 4's O-proj fits in the spare
                    # pump slots of the final (largest) attention section
                    op_queue.append(oproj_gen(4))
                attn_chunk(qg, b, norm_inline=(sc == NSC - 1))
                flush_ops()
            oproj_chunk(NSC - 3)
            oproj_chunk(NSC - 2)
            # keep the PE p-state ramped while the final AllGather drains:
            # a dependency-free dummy chain fills the gap so the last
            # O-projection runs at full clock
            dmy_ps = opps.tile([128, SC_W], fp32, tag="op", name="dmy_ps")
            for i in range(20):
                nc.tensor.matmul(dmy_ps[:], on2_sb[:], cos_sb[:, 0:SC_W],
                                 start=(i == 0), stop=(i == 19))
            dmy_sb = obpool.tile([128, SC_W], bf16, tag="ob", name="dmy_sb")
            nc.scalar.copy(dmy_sb[:], dmy_ps[:])
            nc.sync.dma_start(warm_in_d[0:8, 0:16], dmy_sb[0:8, 0:16])
            oproj7_split()

    nc.compile()
    return nc


def _get_nc(S):
    if S not in _CACHE:
        _CACHE[S] = _build(S)
    return _CACHE[S]


def make_inputs(x, freqs_cis, mask, wq, wk, wv, wo):
    """Host-side sharding / layout prep. Returns in_maps for 8 cores."""
    S = x.shape[1]
    flat_xt = np.ascontiguousarray(np.asarray(x, np.float32).reshape(B * S, DIM).T)
    cos = np.asarray(freqs_cis[..., 0], np.float32)   # [S, HD/2]
    sin = np.asarray(freqs_cis[..., 1], np.float32)
    cos_t = np.ascontiguousarray(np.repeat(cos.T, 2, axis=0))  # [HD, S]
    sin_t = np.ascontiguousarray(np.repeat(sin.T, 2, axis=0))
    m = np.asarray(mask, np.float32)[0, 0]
    nqt = S // 128
    mask_diag = np.ascontiguousarray(
        np.stack([m[i * 128:(i + 1) * 128, i * 128:(i + 1) * 128].T
                  for i in range(nqt)]))
    import ml_dtypes
    bf = ml_dtypes.bfloat16
    flat_xt = flat_xt.astype(bf)
    cos_t = cos_t.astype(bf)
    sin_t = sin_t.astype(bf)
    P = np.zeros((128, 128), np.float32)
    for j in range(64):
        P[2 * j, 2 * j + 1] = -1.0
        P[2 * j + 1, 2 * j] = 1.0
    rotp = np.ascontiguousarray(P.T)

    wq = np.asarray(wq, np.float32)
    wk = np.asarray(wk, np.float32)
    wv = np.asarray(wv, np.float32)
    wo = np.asarray(wo, np.float32)
    in_maps = []
    for c in range(N_CORES):
        r = slice(c * OC, (c + 1) * OC)
        rd = slice(c * OCD, (c + 1) * OCD)
        in_maps.append({
            "xt": flat_xt,
            "wqt": np.ascontiguousarray(wq[r, :].T).astype(bf),
            "wkt": np.ascontiguousarray(wk[r, :].T).astype(bf),
            "wvt": np.ascontiguousarray(wv[r, :].T).astype(bf),
            "wotc": np.ascontiguousarray(wo[rd, :].T).astype(bf),
            "cos_t": cos_t,
            "sin_t": sin_t,
            "mask_diag": mask_diag.astype(bf),
            "rotp": rotp.astype(bf),
            "ones128": np.ones((128, 128), dtype=bf),
        })
    return in_maps


def assemble(results, S):
    """Column-concat per-core output shards into the full output."""
    full = np.empty((B * S, DIM), np.float32)
    for c in range(N_CORES):
        full[:, c * OCD:(c + 1) * OCD] = \
            np.asarray(results[c]["outT"], np.float32).T
    return full.reshape(B, S, DIM)


def kernel(x, start_pos, freqs_cis, mask, wq, wk, wv, wo):
    from concourse.bass_utils import run_bass_kernel_spmd
    S = x.shape[1]
    nc = _get_nc(S)
    in_maps = make_inputs(x, freqs_cis, mask, wq, wk, wv, wo)
    res = run_bass_kernel_spmd(nc, in_maps, core_ids=list(range(N_CORES)))
    return assemble(res.results, S)


# revision 51
# speedup vs baseline: 1.0678x; 1.0678x over previous
"""Trainium2 8-core tensor-parallel attention kernel (Bass/Tile), v3.

Sharding: heads tensor-parallel across 8 cores (2 heads/core) for
QKV + attention; output projection is column-sharded (each core owns
256 output channels) fed by per-chunk AllGathers of the pre-projection
attention outputs (2MB total exchanged vs 16.8MB for post-wo
ReduceScatter).

Single fused loop, interleaved to keep the PE p-state ramped:
  for sc in 0..7:                  # 512 flat seq rows; b = sc//4
    QKV q-chain for h=0
    normalize + AllGather of chunk sc-1   (softmax sums ready by now)
    rest of QKV (Q/K in [hd,seq] + RoPE, V in natural [seq,hd])
    attention q-group (scores/exp/AV; sums via in-place bf16 tree
    reduction on the contiguous probs buffer)
    O-projection of chunk sc-3            (AllGather long since done)

Self-contained: hardcodes B=2, S=2048, DIM=2048, NH=16, HD=128.
"""
import math

import numpy as np

B, S_FULL, DIM, NH = 2, 2048, 2048, 16
HD = 128
N_CORES = 8
HPC = NH // N_CORES          # heads per core (2)
OC = HPC * HD                # q/k/v channels per core (256)
OCD = DIM // N_CORES         # output channels per core (256)
DT = DIM // 128              # dim tiles (16)
SC_W = 512                   # schunk width (cols of flattened seq)

_CACHE = {}


def _build(S):
    """Build the 8-core SPMD Bass graph for sequence length S (B=2 fixed)."""
    import concourse.bass as bass
    import concourse.mybir as mybir
    import concourse.tile as tile
    from concourse import bacc

    fp32 = mybir.dt.float32
    bf16 = mybir.dt.bfloat16
    Exp = mybir.ActivationFunctionType.Exp
    Copy = mybir.ActivationFunctionType.Copy
    BYPASS = mybir.AluOpType.bypass

    FLAT = B * S                 # flattened rows (4096)
    NSC = FLAT // SC_W           # schunks / chunks (8)
    NQT = S // 128               # k-tiles per batch (16)
    SCALE = 1.0 / math.sqrt(HD)
    rg = [list(range(N_CORES))]

    nc = bacc.Bacc("TRN2", target_bir_lowering=False, debug=False,
                   num_devices=N_CORES)

    # ---- external parameters ----
    xt_d = nc.declare_dram_parameter("xt", [DIM, FLAT], bf16, isOutput=False)
    wqt_d = nc.declare_dram_parameter("wqt", [DIM, OC], bf16, isOutput=False)
    wkt_d = nc.declare_dram_parameter("wkt", [DIM, OC], bf16, isOutput=False)
    wvt_d = nc.declare_dram_parameter("wvt", [DIM, OC], bf16, isOutput=False)
    wot_d = nc.declare_dram_parameter("wotc", [DIM, OCD], bf16, isOutput=False)
    cos_d = nc.declare_dram_parameter("cos_t", [HD, S], bf16, isOutput=False)
    sin_d = nc.declare_dram_parameter("sin_t", [HD, S], bf16, isOutput=False)
    mdg_d = nc.declare_dram_parameter("mask_diag", [NQT, 128, 128], bf16, isOutput=False)
    rot_d = nc.declare_dram_parameter("rotp", [128, 128], bf16, isOutput=False)
    on2_d = nc.declare_dram_parameter("ones128", [128, 128], bf16, isOutput=False)
    out_d = nc.declare_dram_parameter("outT", [OCD, FLAT], bf16, isOutput=True)

    # ---- internal DRAM (collective staging) ----
    ag_in_d = [nc.dram_tensor(f"ag_in{p}", [OC, SC_W], bf16) for p in range(NSC)]
    ag_out_d = [nc.dram_tensor(f"ag_out{p}", [N_CORES * OC, SC_W], bf16,
                               addr_space="Shared") for p in range(NSC)]
    warm_in_d = nc.dram_tensor("warm_in", [8, 16], bf16)
    warm_out_d = nc.dram_tensor("warm_out", [64, 16], bf16, addr_space="Shared")

    from contextlib import ExitStack
    with tile.TileContext(nc) as tc:
        with ExitStack() as _stk:
            cpool = _stk.enter_context(tc.tile_pool(name="consts", bufs=1))
            wpool = _stk.enter_context(tc.tile_pool(name="wqkv", bufs=1))
            xpool = _stk.enter_context(tc.tile_pool(name="xT", bufs=34))
            qkpool = _stk.enter_context(tc.tile_pool(name="qk_sb", bufs=1))
            vpool = _stk.enter_context(tc.tile_pool(name="vbf", bufs=1))
            spool = _stk.enter_context(tc.tile_pool(name="p1tmp", bufs=3))
            tpool = _stk.enter_context(tc.tile_pool(name="t1tmp", bufs=2))
            ptpool = _stk.enter_context(tc.tile_pool(name="probsT", bufs=1))
            smpool = _stk.enter_context(tc.tile_pool(name="small", bufs=2))
            aglpool = _stk.enter_context(tc.tile_pool(name="agl", bufs=17))
            obpool = _stk.enter_context(tc.tile_pool(name="outsb", bufs=3))
            qkvps = _stk.enter_context(tc.tile_pool(name="qkvps", bufs=2, space="PSUM"))
            rotps = _stk.enter_context(tc.tile_pool(name="rotps", bufs=1, space="PSUM"))
            scps = _stk.enter_context(tc.tile_pool(name="scps", bufs=2, space="PSUM"))
            pops = _stk.enter_context(tc.tile_pool(name="pops", bufs=2, space="PSUM"))
            opps = _stk.enter_context(tc.tile_pool(name="opps", bufs=1, space="PSUM"))

            # warm-up collective: absorbs the cold-start mesh setup (~40-70us
            # on the first collective) behind the DMA head phase
            nc.gpsimd.collective_compute(
                "AllGather", BYPASS, replica_groups=rg,
                ins=[warm_in_d[:]], outs=[warm_out_d[:]])

            # ---- weights + first x tiles interleaved (critical path) ----
            # per-dt weight tiles so the first chains start as soon as their
            # first slabs land, not after the whole 1MB tensor
            w_sb = {nm: [] for nm in ("q", "k", "v")}
            for nm, d in (("q", wqt_d), ("k", wkt_d), ("v", wvt_d)):
                for dt in range(DT):
                    w = wpool.tile([128, OC], bf16, tag=f"w{nm}{dt}",
                                   name=f"w{nm}{dt}")
                    w_sb[nm].append(w)
            xts0 = []
            for dt in range(DT):
                nc.sync.dma_start(w_sb["q"][dt][:],
                                  wqt_d[dt * 128:(dt + 1) * 128, :])
                xt = xpool.tile([128, SC_W], bf16, tag="xt", name=f"xt{dt}")
                nc.sync.dma_start(xt[:], xt_d[dt * 128:(dt + 1) * 128, 0:SC_W])
                xts0.append(xt)
            for dt in range(DT):
                nc.sync.dma_start(w_sb["k"][dt][:],
                                  wkt_d[dt * 128:(dt + 1) * 128, :])
            for dt in range(DT):
                nc.sync.dma_start(w_sb["v"][dt][:],
                                  wvt_d[dt * 128:(dt + 1) * 128, :])

            # persistent SBUF tensors
            qTa, kTa, vbfa = {}, {}, {}
            for bb in range(B):
                for h in range(HPC):
                    qTa[(bb, h)] = qkpool.tile([128, S], bf16, tag=f"qT{bb}{h}",
                                               name=f"qT{bb}{h}")
                    kTa[(bb, h)] = qkpool.tile([128, S], bf16, tag=f"kT{bb}{h}",
                                               name=f"kT{bb}{h}")
                    vbfa[(bb, h)] = vpool.tile([128, NQT, HD], bf16,
                                               tag=f"v{bb}{h}", name=f"v{bb}{h}")

            cos_sb = cpool.tile([HD, S], bf16)
            sin_sb = cpool.tile([HD, S], bf16)
            mdg_sb = cpool.tile([128, NQT, 128], bf16)
            rot_sb = cpool.tile([128, 128], bf16)
            on2_sb = cpool.tile([128, 128], bf16)
            wot_sb = cpool.tile([128, DT, OCD], bf16)

            def load_consts():
                nc.sync.dma_start(cos_sb[:], cos_d[:])
                nc.sync.dma_start(sin_sb[:], sin_d[:])
                nc.sync.dma_start(mdg_sb[:], mdg_d[:].rearrange("t p k -> p t k"))
                nc.sync.dma_start(rot_sb[:], rot_d[:])
                nc.sync.dma_start(on2_sb[:], on2_d[:])
                for dt in range(DT):
                    nc.sync.dma_start(wot_sb[:, dt, :],
                                      wot_d[dt * 128:(dt + 1) * 128, :])

            # state carried between loop iterations for deferred normalize
            pending = {}   # chunk p -> (qg, b, {h: (pt_buf, po_ps)})
            # pending O-proj work quanta (generators), pumped between
            # attention kt-steps to fill the PE's exp-wait micro-gaps
            op_queue = []

            def pump(n):
                k = 0
                while op_queue and k < n:
                    try:
                        next(op_queue[0])
                        k += 1
                    except StopIteration:
                        op_queue.pop(0)

            def flush_ops():
                while op_queue:
                    try:
                        next(op_queue[0])
                    except StopIteration:
                        op_queue.pop(0)

            # ============ per-chunk attention (h sections only) ============
            def attn_chunk(qg, b):
                kmax = qg * 4 + 3
                K = kmax + 1
                p = b * 4 + qg
                hstate = {}
                for h in range(HPC):
                    po_ps = pops.tile([128, SC_W], fp32, tag="po", name=f"po{h}")
                    ptb = ptpool.tile([128, NQT, SC_W], bf16, tag=f"pTb{h}",
                                      name=f"pTb{h}")
                    # zero the above-diagonal cols of the partial tiles so the
                    # tree reduction sees exact zeros there
                    for kt in range(qg * 4 + 1, kmax + 1):
                        qlo = (kt - qg * 4) * 128
                        nc.gpsimd.memset(ptb[:, kt, 0:qlo], 0)
                    for kt in range(K):
                        qlo = max(0, kt - qg * 4) * 128
                        n = SC_W - qlo
                        sp = scps.tile([128, SC_W], fp32, tag="sc", name="sp")
                        nc.tensor.matmul(
                            sp[:, :n],
                            kTa[(b, h)][:, kt * 128:(kt + 1) * 128],
                            qTa[(b, h)][:, qg * SC_W + qlo:(qg + 1) * SC_W],
                            start=True, stop=True)
                        if kt >= qg * 4:
                            nc.vector.tensor_add(
                                sp[:, 0:128], sp[:, 0:128], mdg_sb[:, kt, :])
                        nc.scalar.activation(ptb[:, kt, qlo:SC_W], sp[:, :n], Exp)
                        if kt >= 1:
                            kl = kt - 1
                            ql2 = max(0, kl - qg * 4) * 128
                            nc.tensor.matmul(
                                po_ps[:, ql2:SC_W], vbfa[(b, h)][:, kl, :],
                                ptb[:, kl, ql2:SC_W],
                                start=(kl == 0), stop=False)
                        pump(2)
                    ql2 = max(0, kmax - qg * 4) * 128
                    nc.tensor.matmul(
                        po_ps[:, ql2:SC_W], vbfa[(b, h)][:, kmax, :],
                        ptb[:, kmax, ql2:SC_W], start=(kmax == 0), stop=True)
                    # in-place bf16 tree reduction over the kt axis -> ptb[:,0,:]
                    kk = K
                    while kk > 1:
                        m = kk // 2
                        nc.vector.tensor_add(ptb[:, 0:m, :], ptb[:, 0:m, :],
                                             ptb[:, kk - m:kk, :])
                        kk -= m
                    hstate[h] = (ptb, po_ps)
                pending[p] = (qg, b, hstate)

            # ============ deferred normalize + AllGather ============
            def normalize_chunk(p):
                qg, b, hstate = pending.pop(p)
                for h in range(HPC):
                    ptb, po_ps = hstate[h]
                    sb_ps = rotps.tile([128, SC_W], fp32, tag="rot", name="sb_ps")
                    nc.tensor.matmul(sb_ps[:], on2_sb[:], ptb[:, 0, :],
                                     start=True, stop=True)
                    rbc = smpool.tile([128, SC_W], fp32, tag="rbc", name="rbc")
                    nc.vector.reciprocal_approx_fast(rbc[:], sb_ps[:])
                    ob = smpool.tile([128, SC_W], bf16, tag="obuf", name="ob")
                    nc.vector.tensor_mul(ob[:], po_ps[:], rbc[:])
                    nc.sync.dma_start(ag_in_d[p][h * 128:(h + 1) * 128, :], ob[:])
                nc.gpsimd.collective_compute(
                    "AllGather", BYPASS, replica_groups=rg,
                    ins=[ag_in_d[p][:]], outs=[ag_out_d[p][:]])

            # ============ per-chunk O-projection (column-sharded) ============
            def oproj_gen(p):
                """Generator: one yield per matmul so the caller can weave
                the chain between attention steps."""
                slabs = []
                for e in range(DT):
                    agl = aglpool.tile([128, SC_W], bf16, tag="agl",
                                       name=f"agl{e}")
                    nc.sync.dma_start(agl[:],
                                      ag_out_d[p][e * 128:(e + 1) * 128, :])
                    slabs.append(agl)
                for dh in range(2):
                    # dh=1 borrows the rot pool's bank so the two half-chains
                    # don't serialize on a single PSUM drain
                    pool = opps if dh == 0 else rotps
                    op_ps = pool.tile([128, SC_W], fp32,
                                      tag="op" if dh == 0 else "rot",
                                      name="op_ps")
                    for e in range(DT):
                        nc.tensor.matmul(
                            op_ps[:],
                            wot_sb[:, e, dh * 128:(dh + 1) * 128],
                            slabs[e][:],
                            start=(e == 0), stop=(e == DT - 1))
                        yield
                    obt = obpool.tile([128, SC_W], bf16, tag="ob", name="obt")
                    if dh == 0:
                        nc.scalar.copy(obt[:], op_ps[:])
                    else:
                        nc.vector.tensor_copy(obt[:], op_ps[:])
                    nc.sync.dma_start(
                        out_d[dh * 128:(dh + 1) * 128,
                              p * SC_W:(p + 1) * SC_W], obt[:])

            def oproj_chunk(p):
                for _ in oproj_gen(p):
                    pass

            # ============ main fused loop ============
            carried_xts = None
            for sc in range(NSC):
                b, qg = divmod(sc, 4)
                c0 = qg * SC_W               # column offset within batch
                xts = xts0 if sc == 0 else carried_xts

                def emit_rope_mm(t, h, til):
                    rp = rotps.tile([128, SC_W], fp32, tag="rot", name="rp")
                    nc.tensor.matmul(rp[:], rot_sb[:], til[:],
                                     start=True, stop=True)
                    dst = (qTa if t == "q" else kTa)[(b, h)]
                    t1 = tpool.tile([128, SC_W], bf16, tag="t1", name="t1")
                    nc.gpsimd.tensor_mul(t1[:], til[:], cos_sb[:, c0:c0 + SC_W])
                    hat = spool.tile([128, SC_W], bf16, tag="hat", name="hat")
                    nc.vector.tensor_mul(hat[:], rp[:], sin_sb[:, c0:c0 + SC_W])
                    nc.vector.tensor_add(dst[:, c0:c0 + SC_W], hat[:], t1[:])

                def v_gen(vsc, vxts):
                    """Natural-layout V chains for schunk vsc (stationary =
                    x tile, both heads at once). No scalar inputs, so these
                    weave safely into exp-bound attention sections."""
                    vb, vqg = divmod(vsc, 4)
                    for vt in range(4):
                        psv = qkvps.tile([128, OC], fp32, tag="qkv",
                                         name="ps_v")
                        for dt in range(DT):
                            nc.tensor.matmul(
                                psv[:],
                                vxts[dt][:, vt * 128:(vt + 1) * 128],
                                w_sb["v"][dt][:],
                                start=(dt == 0), stop=(dt == DT - 1))
                            yield
                        for hh in range(HPC):
                            if hh == 0:
                                nc.scalar.copy(
                                    vbfa[(vb, hh)][:, vqg * 4 + vt, :],
                                    psv[:, hh * HD:(hh + 1) * HD])
                            else:
                                nc.vector.tensor_copy(
                                    vbfa[(vb, hh)][:, vqg * 4 + vt, :],
                                    psv[:, hh * HD:(hh + 1) * HD])

                first_chain_done = False
                for h in range(HPC):
                    tils = {}
                    for t in ("q", "k"):
                        ps = qkvps.tile([128, SC_W], fp32, tag="qkv",
                                        name=f"ps_{t}")
                        for dt in range(DT):
                            nc.tensor.matmul(
                                ps[:],
                                w_sb[t][dt][:, h * HD:(h + 1) * HD],
                                xts[dt][:],
                                start=(dt == 0), stop=(dt == DT - 1))
                        if not first_chain_done:
                            first_chain_done = True
                            if sc == 0:
                                load_consts()
                            if sc >= 1:
                                # normalize previous chunk while this chunk's
                                # q-chain covers the PE
                                normalize_chunk(sc - 1)
                        til = spool.tile([128, SC_W], bf16, tag="til",
                                         name=f"til_{t}")
                        if t == "q":
                            nc.scalar.activation(til[:], ps[:], Copy,
                                                 scale=SCALE)
                        else:
                            nc.scalar.copy(til[:], ps[:])
                        tils[t] = til
                        if t == "k":
                            emit_rope_mm("q", h, tils["q"])
                    if h == 0 and sc == 0:
                        # sc 0's V runs inline; later scs' V is woven into
                        # the previous chunk's attention
                        for _ in v_gen(0, xts):
                            pass
                    emit_rope_mm("k", h, tils["k"])

                # prefetch next schunk's x tiles for the woven V chains
                if sc + 1 < NSC:
                    nxts = []
                    for dt in range(DT):
                        xt = xpool.tile([128, SC_W], bf16, tag="xt",
                                        name=f"xt{dt}")
                        nc.sync.dma_start(
                            xt[:], xt_d[dt * 128:(dt + 1) * 128,
                                        (sc + 1) * SC_W:(sc + 2) * SC_W])
                        nxts.append(xt)
                    next_xts = nxts
                else:
                    next_xts = None
                carried_xts = next_xts

                # weave next schunk's V and earlier chunks' O-proj through
                # this chunk's attention (exp-bound) section
                if next_xts is not None:
                    op_queue.append(v_gen(sc + 1, next_xts))
                if sc >= 4:
                    op_queue.append(oproj_gen(sc - 4))
                attn_chunk(qg, b)
                flush_ops()
            normalize_chunk(NSC - 1)
            oproj_chunk(NSC - 4)
            oproj_chunk(NSC - 3)
            oproj_chunk(NSC - 2)
            # keep the PE p-state ramped while the final AllGather drains:
            # a dependency-free dummy chain fills the gap so the last
            # O-projection runs at full clock
            dmy_ps = opps.tile([128, SC_W], fp32, tag="op", name="dmy_ps")
            for i in range(20):
                nc.tensor.matmul(dmy_ps[:], on2_sb[:], cos_sb[:, 0:SC_W],
                                 start=(i == 0), stop=(i == 19))
            dmy_sb = obpool.tile([128, SC_W], bf16, tag="ob", name="dmy_sb")
            nc.scalar.copy(dmy_sb[:], dmy_ps[:])
            nc.sync.dma_start(warm_in_d[0:8, 0:16], dmy_sb[0:8, 0:16])
            oproj_chunk(NSC - 1)

    nc.compile()
    return nc


def _get_nc(S):
    if S not in _CACHE:
        _CACHE[S] = _build(S)
    return _CACHE[S]


def make_inputs(x, freqs_cis, mask, wq, wk, wv, wo):
    """Host-side sharding / layout prep. Returns in_maps for 8 cores."""
    S = x.shape[1]
    flat_xt = np.ascontiguousarray(np.asarray(x, np.float32).reshape(B * S, DIM).T)
    cos = np.asarray(freqs_cis[..., 0], np.float32)   # [S, HD/2]
    sin = np.asarray(freqs_cis[..., 1], np.float32)
    cos_t = np.ascontiguousarray(np.repeat(cos.T, 2, axis=0))  # [HD, S]
    sin_t = np.ascontiguousarray(np.repeat(sin.T, 2, axis=0))
    m = np.asarray(mask, np.float32)[0, 0]
    nqt = S // 128
    mask_diag = np.ascontiguousarray(
        np.stack([m[i * 128:(i + 1) * 128, i * 128:(i + 1) * 128].T
                  for i in range(nqt)]))
    import ml_dtypes
    bf = ml_dtypes.bfloat16
    flat_xt = flat_xt.astype(bf)
    cos_t = cos_t.astype(bf)
    sin_t = sin_t.astype(bf)
    P = np.zeros((128, 128), np.float32)
    for j in range(64):
        P[2 * j, 2 * j + 1] = -1.0
        P[2 * j + 1, 2 * j] = 1.0
    rotp = np.ascontiguousarray(P.T)

    wq = np.asarray(wq, np.float32)
    wk = np.asarray(wk, np.float32)
    wv = np.asarray(wv, np.float32)
    wo = np.asarray(wo, np.float32)
    in_maps = []
    for c in range(N_CORES):
        r = slice(c * OC, (c + 1) * OC)
        rd = slice(c * OCD, (c + 1) * OCD)
        in_maps.append({
            "xt": flat_xt,
            "wqt": np.ascontiguousarray(wq[r, :].T).astype(bf),
            "wkt": np.ascontiguousarray(wk[r, :].T).astype(bf),
            "wvt": np.ascontiguousarray(wv[r, :].T).astype(bf),
            "wotc": np.ascontiguousarray(wo[rd, :].T).astype(bf),
            "cos_t": cos_t,
            "sin_t": sin_t,
            "mask_diag": mask_diag.astype(bf),
            "rotp": rotp.astype(bf),
            "ones128": np.ones((128, 128), dtype=bf),
        })
    return in_maps


def assemble(results, S):
    """Column-concat per-core output shards into the full output."""
    full = np.empty((B * S, DIM), np.float32)
    for c in range(N_CORES):
        full[:, c * OCD:(c + 1) * OCD] = \
            np.asarray(results[c]["outT"], np.float32).T
    return full.reshape(B, S, DIM)


def kernel(x, start_pos, freqs_cis, mask, wq, wk, wv, wo):
    from concourse.bass_utils import run_bass_kernel_spmd
    S = x.shape[1]
    nc = _get_nc(S)
    in_maps = make_inputs(x, freqs_cis, mask, wq, wk, wv, wo)
    res = run_bass_kernel_spmd(nc, in_maps, core_ids=list(range(N_CORES)))
    return assemble(res.results, S)
